# revision 11
# baseline (speedup 1.0000x reference)
"""Trainium2 Bass kernel for nn_DeepseekOcrImageTokenScatterBlock.

Reference semantics (B=4, S=4096, H=2048, N_IMG=B*S):
    mask  = images_seq_mask.reshape(-1)               # [T] bool, T = B*S
    ranks = cumsum(mask) - 1                          # global masked-token rank
    out[t] = images_in_this_batch[ranks[t]] if mask[t] else inputs_embeds[t]

Strategy (8-way SPMD, 2048 tokens per core):
  Host side (layout only): per core c, build a gather table
  [embeds rows 2048c..2048c+2047 ; all 16384 images rows] = [18432, 2048] f32,
  the full mask as a [128, 128] uint8 grid (token t = p*128 + f), a one-hot
  selection matrix picking this core's 16 grid rows, and an affine iota
  constant (2047 - local_t over the global grid).

  Device side (all arithmetic): per-partition inclusive prefix-scan of the
  mask (tensor_tensor_scan), close the scan across partitions with a
  strict-upper-triangular matmul, fuse in the iota constant, zero unmasked
  entries, then a single one-hot matmul that simultaneously selects this
  core's rows AND transposes, yielding per-token gather rows
    idx = local_t           if unmasked   (table rows 0..2047)
        = cumsum + 2047     if masked     (2048 + rank, table rows 2048..)
  as [128, 16] int32. Then 16 indirect row-gather DMAs (128 rows x 8 KiB
  each) from the table into SBUF and 16 stores to the output slice. Per-core
  HBM traffic is the 32 MiB minimum (16 MiB read + 16 MiB write).
"""

import sys

import numpy as np

for _p in ("/opt/trn_rl_repo",):
    if _p not in sys.path:
        sys.path.insert(0, _p)

import concourse.bass as bass
import concourse.tile as tile
from concourse import mybir
from concourse.bass_utils import run_bass_kernel_spmd
from concourse.masks import make_upper_triangular

B, S, H = 4, 4096, 2048
T = B * S  # 16384 tokens
N_CORES = 8
TPC = T // N_CORES  # 2048 tokens per core
P = 128  # partitions
FCOLS = T // P  # 128 free columns in the mask grid (token t = p*128 + f)
BLK = TPC // P  # 16 grid rows (and gather tiles) per core
TABLE_ROWS = TPC + T  # 18432

# ---------------------------------------------------------------------------
# The walrus build in this container rejects instructions carrying more than
# one sync-wait ("Too many sync wait commands" in codegen setupSyncWait).
# Tile's semaphore assignment freely attaches several waits to one
# instruction, so after tracing we split: each extra wait moves onto its own
# single-wait NOP inserted just before the instruction on the same engine.
# Per-engine program order makes this semantically identical.
_wsplit_counter = [0]


def _split_multi_waits(nc, max_waits=1):
    for fn in nc.m.functions:
        for blk in fn.blocks:
            insts = blk.instructions
            out = []
            changed = False
            for inst in insts:
                si = inst.sync_info
                waits = list(si.on_wait) if (si is not None and si.on_wait) else []
                if len(waits) > max_waits:
                    changed = True
                    for w in waits[:-max_waits]:
                        _wsplit_counter[0] += 1
                        nop = mybir.InstNoOp(
                            name=f"I-wsplit-{_wsplit_counter[0]}", ins=[], outs=[]
                        )
                        nop.engine = inst.engine
                        nop.sync_info = type(si)(on_wait=[w], on_update=[])
                        nc.register_instruction(nop, overwrite=True)
                        out.append(nop)
                    si.on_wait = waits[-max_waits:]
                out.append(inst)
            if changed:
                blk.instructions = out
# ---------------------------------------------------------------------------


def _build_nc():
    nc = bass.Bass("TRN2", target_bir_lowering=False, debug=False, num_devices=N_CORES)
    mask_d = nc.dram_tensor("mask", [P, FCOLS], mybir.dt.uint8, kind="ExternalInput")
    sel_d = nc.dram_tensor("sel", [P, BLK], mybir.dt.float32, kind="ExternalInput")
    iota2_d = nc.dram_tensor(
        "iota2", [P, FCOLS], mybir.dt.float32, kind="ExternalInput"
    )
    table_d = nc.dram_tensor(
        "table", [TABLE_ROWS, H], mybir.dt.float32, kind="ExternalInput"
    )
    out_d = nc.dram_tensor("out", [TPC, H], mybir.dt.float32, kind="ExternalOutput")

    f32 = mybir.dt.float32
    with tile.TileContext(nc) as tc:
        with (
            tc.tile_pool(name="sbuf", bufs=1) as sp,
            tc.tile_pool(name="psum", bufs=1, space="PSUM") as pp,
            tc.tile_pool(name="gather", bufs=6) as gp,
        ):
            # Input loads first — the mask DMA heads the critical path.
            mask_sb = sp.tile([P, FCOLS], mybir.dt.uint8)
            nc.sync.dma_start(mask_sb[:], mask_d.ap()[:, :])
            sel_sb = sp.tile([P, BLK], f32)
            nc.scalar.dma_start(sel_sb[:], sel_d.ap()[:, :])
            iota2_sb = sp.tile([P, FCOLS], f32)
            nc.scalar.dma_start(iota2_sb[:], iota2_d.ap()[:, :])

            # Constants (device-generated, off the critical path).
            ustrict = sp.tile([P, P], f32)
            make_upper_triangular(nc, ustrict[:], val=1.0, diag=False)
            # lgrid[f, j] = j*128 + f = this core's local token id of gather
            # tile j, partition f. f32 iota is exact for values < 2^24.
            lgrid = sp.tile([P, BLK], f32)
            nc.gpsimd.iota(
                lgrid[:],
                pattern=[[P, BLK]],
                base=0,
                channel_multiplier=1,
                allow_small_or_imprecise_dtypes=True,
            )
            # Mask cast to f32 on gpsimd, in parallel with the DVE scan.
            maskf = sp.tile([P, FCOLS], f32)
            nc.gpsimd.tensor_copy(maskf[:], mask_sb[:])

            # Global inclusive cumsum over token order t = p*128 + f:
            # per-partition scan along f, then close across partitions with a
            # strict-upper-triangular matmul of the per-partition totals.
            cs = sp.tile([P, FCOLS], f32)
            nc.vector.tensor_tensor_scan(
                out=cs[:],
                data0=mask_sb[:],
                data1=mask_sb[:],
                initial=0.0,
                op0=mybir.AluOpType.add,
                op1=mybir.AluOpType.bypass,
            )
            rowoff_ps = pp.tile([P, 1], f32)
            nc.tensor.matmul(
                rowoff_ps[:],
                lhsT=ustrict[:],
                rhs=cs[:, FCOLS - 1 : FCOLS],
                start=True,
                stop=True,
            )
            # a = cs + rowoff + (2047 - local_t); b = a * mask.  At this core's
            # tokens: b = cs_global + 2047 - local_t if masked else 0.
            a = sp.tile([P, FCOLS], f32)
            nc.vector.scalar_tensor_tensor(
                out=a[:],
                in0=cs[:],
                scalar=rowoff_ps[:, 0:1],
                in1=iota2_sb[:],
                op0=mybir.AluOpType.add,
                op1=mybir.AluOpType.add,
            )
            b = sp.tile([P, FCOLS], f32)
            nc.vector.tensor_tensor(
                out=b[:], in0=a[:], in1=maskf[:], op=mybir.AluOpType.mult
            )
            # One matmul both selects this core's rows and transposes:
            # idxT_ps[f, j] = sum_p b[p, f] * sel[p, j].  Adding lgrid restores
            # the unmasked local id and cancels the masked -local_t, leaving
            #   idxT = local_t (unmasked) | cs_global + 2047 (masked).
            idxT_ps = pp.tile([P, BLK], f32)
            nc.tensor.matmul(
                idxT_ps[:], lhsT=b[:], rhs=sel_sb[:], start=True, stop=True
            )
            idxT = sp.tile([P, BLK], mybir.dt.int32)
            nc.vector.tensor_tensor(
                out=idxT[:], in0=idxT_ps[:], in1=lgrid[:], op=mybir.AluOpType.add
            )

            # Main data movement: 16 x (indirect row gather + store), 1 MiB each.
            for j in range(BLK):
                g = gp.tile([P, H], f32, tag="g")
                nc.gpsimd.indirect_dma_start(
                    out=g[:],
                    out_offset=None,
                    in_=table_d.ap()[:, :],
                    in_offset=bass.IndirectOffsetOnAxis(
                        ap=idxT[:, j : j + 1], axis=0
                    ),
                )
                store_eng = nc.sync if j % 2 == 0 else nc.scalar
                store_eng.dma_start(out_d.ap()[j * P : (j + 1) * P, :], g[:])

    _split_multi_waits(nc)
    return nc


_NC = None
_RUN_KWARGS: dict = {}
_LAST_RESULTS = None


def _get_nc():
    global _NC
    if _NC is None:
        _NC = _build_nc()
    return _NC


def kernel(inputs_embeds, images_seq_mask, images_in_this_batch):
    global _LAST_RESULTS
    emb = np.ascontiguousarray(np.asarray(inputs_embeds, dtype=np.float32)).reshape(
        T, H
    )
    images = np.ascontiguousarray(np.asarray(images_in_this_batch, dtype=np.float32))
    mask_grid = np.ascontiguousarray(
        np.asarray(images_seq_mask).reshape(T).astype(np.uint8).reshape(P, FCOLS)
    )

    t_global = (np.arange(P)[:, None] * FCOLS + np.arange(FCOLS)[None, :]).astype(
        np.int64
    )
    in_maps = []
    for c in range(N_CORES):
        sel = np.zeros((P, BLK), np.float32)
        sel[np.arange(BLK) + c * BLK, np.arange(BLK)] = 1.0
        iota2 = ((TPC - 1) - (t_global - c * TPC)).astype(np.float32)
        table = np.concatenate([emb[c * TPC : (c + 1) * TPC], images], axis=0)
        in_maps.append(
            {"mask": mask_grid, "sel": sel, "iota2": iota2, "table": table}
        )

    for attempt in range(3):
        try:
            res = run_bass_kernel_spmd(
                _get_nc(), in_maps, core_ids=list(range(N_CORES)), **_RUN_KWARGS
            )
            break
        except Exception:  # transient axon/NRT faults (device wedge)
            if attempt == 2:
                raise
            import time as _time

            _time.sleep(10.0 * (attempt + 1))
    _LAST_RESULTS = res
    out = np.concatenate([res.results[c]["out"] for c in range(N_CORES)], axis=0)
    return out.reshape(B, S, H)
